# revision 3
# baseline (speedup 1.0000x reference)
"""Concordance CC (segment_reduce) Trainium2 Bass kernel.

Problem: y_true, y_pred [256, 65536] f32, prefix-validity mask [256, 65536] i32.
Per row: masked means/variances/covariance (ddof=1), ccc = 2*cov /
(var_t + var_p + 2*(mean_t - mean_p)); output = mean(ccc) (scalar f32).

Strategy (data parallel over B, 8 cores x 32 rows):
Every per-row statistic is an inner product over T of columns from
W = [a, b, m] with a = y_true*mask, b = y_pred*mask, m = mask:
  S2t=a.a  Stp=a.b  S1t=a.m  S2p=b.b  S1p=b.m  L=m.m
So each core computes one 96x96 Gram matrix W^T W (W has 3 cols per row,
32 rows) on the TensorEngine, accumulating over T in PSUM. The VectorEngine
only builds the masked fp16 operands; DMA is the roofline (~25 MB/core).
Host does the O(B) scalar epilogue.
"""

import numpy as np

import concourse.bass as bass
import concourse.tile as tile
from concourse import mybir
from concourse.bass_utils import run_bass_kernel_spmd

# ---------------------------------------------------------------- constants
B, T = 256, 65536
NCORES = 8
R = B // NCORES            # rows per core = 32
TB = 16384                 # T-block size
NBLK = T // TB             # 4
JB = TB // 128             # free elems per row per block = 128
GCOLS = 3 * R              # 96 Gram columns: [a_0..a_31, b_0..b_31, m_0..m_31]
GFREE = GCOLS * JB         # 12288 free elems in the G tile

FP = mybir.dt.float16      # Gram operand precision


def split_multi_waits(nc: bass.Bass) -> int:
    """This container's walrus build accepts at most ONE sync-wait per
    instruction, but Tile's sem assignment attaches all required waits to
    the consuming instruction. Hoist the excess onto same-engine NoOps
    inserted immediately before it (sequencers execute in order, so the
    waits are still satisfied before the instruction issues)."""
    n_split = 0
    for f in nc.m.functions:
        for bb in f.blocks:
            insts = bb.instructions
            out = []
            for inst in insts:
                si = inst.sync_info
                if si is not None and si.on_wait and len(si.on_wait) > 1:
                    waits = list(si.on_wait)
                    for w in waits[:-1]:
                        nop = mybir.InstNoOp(
                            name=f"I-wsplit-{nc.next_id()}", ins=[], outs=[]
                        )
                        nop.engine = inst.engine
                        nop.sync_info = mybir.SyncInfo(on_wait=[w], on_update=[])
                        out.append(nop)
                        n_split += 1
                    inst.sync_info = mybir.SyncInfo(
                        on_wait=[waits[-1]], on_update=list(si.on_update or [])
                    )
                out.append(inst)
            bb.instructions = out
    return n_split


def build_nc() -> bass.Bass:
    nc = bass.Bass()
    yt = nc.dram_tensor("y_true", [R, T], mybir.dt.float32, kind="ExternalInput")
    yp = nc.dram_tensor("y_pred", [R, T], mybir.dt.float32, kind="ExternalInput")
    mk = nc.dram_tensor("mask", [R, T], mybir.dt.int32, kind="ExternalInput")
    gram = nc.dram_tensor("gram", [GCOLS, GCOLS], mybir.dt.float32,
                          kind="ExternalOutput")

    with tile.TileContext(nc) as tc:
        with (
            tc.tile_pool(name="gpool", bufs=2) as gpool,
            tc.tile_pool(name="stage", bufs=2) as stage,
            tc.tile_pool(name="psum", bufs=1, space="PSUM") as psum,
            tc.tile_pool(name="outp", bufs=1) as outp,
        ):
            ps = psum.tile([GCOLS, GCOLS], mybir.dt.float32)
            for tb in range(NBLK):
                g = gpool.tile([128, GFREE], FP)
                tt = stage.tile([128, R * JB], mybir.dt.float32)
                tp = stage.tile([128, R * JB], mybir.dt.float32)
                tm = stage.tile([128, R * JB], mybir.dt.int32)

                lo, hi = tb * TB, (tb + 1) * TB
                src = lambda h: h[:, lo:hi].rearrange("r (p j) -> p r j", p=128)
                nc.sync.dma_start(
                    out=tt[:, :].rearrange("p (r j) -> p r j", r=R), in_=src(yt)
                )
                nc.scalar.dma_start(
                    out=tp[:, :].rearrange("p (r j) -> p r j", r=R), in_=src(yp)
                )
                nc.sync.dma_start(
                    out=tm[:, :].rearrange("p (r j) -> p r j", r=R), in_=src(mk)
                )

                ga = g[:, 0 : R * JB]
                gb = g[:, R * JB : 2 * R * JB]
                gm = g[:, 2 * R * JB : 3 * R * JB]
                # m (i32 -> f16), then a = y_true*m, b = y_pred*m
                nc.vector.tensor_copy(out=gm, in_=tm[:, :])
                nc.vector.tensor_mul(out=ga, in0=tt[:, :], in1=gm)
                nc.vector.tensor_mul(out=gb, in0=tp[:, :], in1=gm)

                gv = g[:, :].rearrange("p (k j) -> p k j", j=JB)
                for ci in range(JB):
                    w = gv[:, :, ci]
                    nc.tensor.matmul(
                        ps[:, :],
                        lhsT=w,
                        rhs=w,
                        start=(tb == 0 and ci == 0),
                        stop=(tb == NBLK - 1 and ci == JB - 1),
                    )

            out_t = outp.tile([GCOLS, GCOLS], mybir.dt.float32)
            nc.vector.tensor_copy(out=out_t[:, :], in_=ps[:, :])
            nc.sync.dma_start(out=gram[:, :], in_=out_t[:, :])
    split_multi_waits(nc)
    return nc


_NC_CACHE = None


def _get_nc():
    global _NC_CACHE
    if _NC_CACHE is None:
        _NC_CACHE = build_nc()
    return _NC_CACHE


def _ccc_from_grams(grams: list[np.ndarray]) -> np.ndarray:
    idx = np.arange(R)
    total = 0.0
    for g in grams:
        g = g.astype(np.float64)
        s2t = g[idx, idx]
        stp = g[idx, R + idx]
        s1t = g[idx, 2 * R + idx]
        s2p = g[R + idx, R + idx]
        s1p = g[R + idx, 2 * R + idx]
        ell = g[2 * R + idx, 2 * R + idx]
        mean_t = s1t / ell
        mean_p = s1p / ell
        denom = ell - 1.0
        var_t = (s2t - s1t * s1t / ell) / denom
        var_p = (s2p - s1p * s1p / ell) / denom
        cov = (stp - s1t * s1p / ell) / denom
        ccc = 2.0 * cov / (var_t + var_p + (mean_t - mean_p) * 2.0)
        total += ccc.sum()
    return np.float32(total / B)


def kernel(y_true, y_pred, mask) -> np.ndarray:
    y_true = np.ascontiguousarray(np.asarray(y_true, dtype=np.float32))
    y_pred = np.ascontiguousarray(np.asarray(y_pred, dtype=np.float32))
    mask = np.ascontiguousarray(np.asarray(mask, dtype=np.int32))

    nc = _get_nc()
    in_maps = [
        {
            "y_true": y_true[c * R : (c + 1) * R],
            "y_pred": y_pred[c * R : (c + 1) * R],
            "mask": mask[c * R : (c + 1) * R],
        }
        for c in range(NCORES)
    ]
    res = run_bass_kernel_spmd(nc, in_maps, core_ids=list(range(NCORES)))
    grams = [res.results[c]["gram"] for c in range(NCORES)]
    return _ccc_from_grams(grams)


# revision 4
# speedup vs baseline: 1.2558x; 1.2558x over previous
"""Concordance CC (segment_reduce) Trainium2 Bass kernel.

Problem: y_true, y_pred [256, 65536] f32, prefix-validity mask [256, 65536] i32.
Per row: masked means/variances/covariance (ddof=1), ccc = 2*cov /
(var_t + var_p + 2*(mean_t - mean_p)); output = mean(ccc) (scalar f32).

Strategy (data parallel over B, 8 cores x 32 rows):
Every per-row statistic is an inner product over T of columns from
W = [a, b, m] with a = y_true*mask, b = y_pred*mask, m = mask:
  S2t=a.a  Stp=a.b  S1t=a.m  S2p=b.b  S1p=b.m  L=m.m
So each core computes one 96x96 Gram matrix W^T W (W has 3 cols per row,
32 rows) on the TensorEngine, accumulating over T in PSUM. The VectorEngine
only builds the masked fp16 operands; DMA is the roofline (~25 MB/core).
Host does the O(B) scalar epilogue.
"""

import numpy as np

import concourse.bass as bass
import concourse.tile as tile
from concourse import mybir
from concourse.bass_utils import run_bass_kernel_spmd

# ---------------------------------------------------------------- constants
B, T = 256, 65536
NCORES = 8
R = B // NCORES            # rows per core = 32
TB = 16384                 # T-block size
NBLK = T // TB             # 4
JB = TB // 128             # free elems per row per block = 128
GCOLS = 3 * R              # 96 Gram columns: [a_0..a_31, b_0..b_31, m_0..m_31]
GFREE = GCOLS * JB         # 12288 free elems in the G tile

FP = mybir.dt.float16      # Gram operand precision


def split_multi_waits(nc: bass.Bass) -> int:
    """This container's walrus build accepts at most ONE sync-wait per
    instruction, but Tile's sem assignment attaches all required waits to
    the consuming instruction. Hoist the excess onto same-engine NoOps
    inserted immediately before it (sequencers execute in order, so the
    waits are still satisfied before the instruction issues)."""
    n_split = 0
    for f in nc.m.functions:
        for bb in f.blocks:
            insts = bb.instructions
            out = []
            for inst in insts:
                si = inst.sync_info
                if si is not None and si.on_wait and len(si.on_wait) > 1:
                    waits = list(si.on_wait)
                    for w in waits[:-1]:
                        nop = mybir.InstNoOp(
                            name=f"I-wsplit-{nc.next_id()}", ins=[], outs=[]
                        )
                        nop.engine = inst.engine
                        nop.sync_info = mybir.SyncInfo(on_wait=[w], on_update=[])
                        out.append(nop)
                        n_split += 1
                    inst.sync_info = mybir.SyncInfo(
                        on_wait=[waits[-1]], on_update=list(si.on_update or [])
                    )
                out.append(inst)
            bb.instructions = out
    return n_split


def build_nc() -> bass.Bass:
    nc = bass.Bass()
    yt = nc.dram_tensor("y_true", [R, T], mybir.dt.float32, kind="ExternalInput")
    yp = nc.dram_tensor("y_pred", [R, T], mybir.dt.float32, kind="ExternalInput")
    mk = nc.dram_tensor("mask", [R, T], mybir.dt.int32, kind="ExternalInput")
    gram = nc.dram_tensor("gram", [GCOLS, GCOLS], mybir.dt.float32,
                          kind="ExternalOutput")

    with tile.TileContext(nc) as tc:
        with (
            tc.tile_pool(name="gpool", bufs=2) as gpool,
            tc.tile_pool(name="stage", bufs=2) as stage,
            tc.tile_pool(name="psum", bufs=1, space="PSUM") as psum,
            tc.tile_pool(name="outp", bufs=1) as outp,
        ):
            ps = psum.tile([GCOLS, GCOLS], mybir.dt.float32)
            for tb in range(NBLK):
                # G is chunk-major: G[p, ci*GCOLS + k] so each matmul chunk's
                # operand G[:, ci*96:(ci+1)*96] is contiguous (strided PE APs
                # measured ~8x slower). The DVE writes are strided instead
                # (it runs 1x REGULAR mode either way).
                g = gpool.tile([128, GFREE], FP)
                tt = stage.tile([128, R * JB], mybir.dt.float32)
                tp = stage.tile([128, R * JB], mybir.dt.float32)
                tm = stage.tile([128, R * JB], mybir.dt.int32)

                lo, hi = tb * TB, (tb + 1) * TB
                # staging layout: tile[p, r*JB + c] = src[r, lo + p*JB + c]
                src = lambda h: h[:, lo:hi].rearrange("r (p c) -> p r c", p=128)
                dst = lambda t_: t_[:, :].rearrange("p (r c) -> p r c", r=R)
                # one tensor per DMA path: SP-HWDGE, ACT-HWDGE, SWDGE
                nc.sync.dma_start(out=dst(tt), in_=src(yt))
                nc.scalar.dma_start(out=dst(tp), in_=src(yp))
                nc.gpsimd.dma_start(out=dst(tm), in_=src(mk))

                gv = g[:, :].rearrange("p (c k) -> p c k", k=GCOLS)
                ga = gv[:, :, 0:R]
                gb = gv[:, :, R : 2 * R]
                gm = gv[:, :, 2 * R : 3 * R]
                stg = lambda t_: t_[:, :].rearrange("p (r c) -> p c r", r=R)
                # m (i32 -> f16), then a = y_true*m, b = y_pred*m
                nc.vector.tensor_copy(out=gm, in_=stg(tm))
                nc.vector.tensor_mul(out=ga, in0=stg(tt), in1=gm)
                nc.vector.tensor_mul(out=gb, in0=stg(tp), in1=gm)

                for ci in range(JB):
                    w = g[:, ci * GCOLS : (ci + 1) * GCOLS]
                    nc.tensor.matmul(
                        ps[:, :],
                        lhsT=w,
                        rhs=w,
                        start=(tb == 0 and ci == 0),
                        stop=(tb == NBLK - 1 and ci == JB - 1),
                    )

            out_t = outp.tile([GCOLS, GCOLS], mybir.dt.float32)
            nc.vector.tensor_copy(out=out_t[:, :], in_=ps[:, :])
            nc.sync.dma_start(out=gram[:, :], in_=out_t[:, :])
    split_multi_waits(nc)
    return nc


_NC_CACHE = None


def _get_nc():
    global _NC_CACHE
    if _NC_CACHE is None:
        _NC_CACHE = build_nc()
    return _NC_CACHE


def _ccc_from_grams(grams: list[np.ndarray]) -> np.ndarray:
    idx = np.arange(R)
    total = 0.0
    for g in grams:
        g = g.astype(np.float64)
        s2t = g[idx, idx]
        stp = g[idx, R + idx]
        s1t = g[idx, 2 * R + idx]
        s2p = g[R + idx, R + idx]
        s1p = g[R + idx, 2 * R + idx]
        ell = g[2 * R + idx, 2 * R + idx]
        mean_t = s1t / ell
        mean_p = s1p / ell
        denom = ell - 1.0
        var_t = (s2t - s1t * s1t / ell) / denom
        var_p = (s2p - s1p * s1p / ell) / denom
        cov = (stp - s1t * s1p / ell) / denom
        ccc = 2.0 * cov / (var_t + var_p + (mean_t - mean_p) * 2.0)
        total += ccc.sum()
    return np.float32(total / B)


def kernel(y_true, y_pred, mask) -> np.ndarray:
    y_true = np.ascontiguousarray(np.asarray(y_true, dtype=np.float32))
    y_pred = np.ascontiguousarray(np.asarray(y_pred, dtype=np.float32))
    mask = np.ascontiguousarray(np.asarray(mask, dtype=np.int32))

    nc = _get_nc()
    in_maps = [
        {
            "y_true": y_true[c * R : (c + 1) * R],
            "y_pred": y_pred[c * R : (c + 1) * R],
            "mask": mask[c * R : (c + 1) * R],
        }
        for c in range(NCORES)
    ]
    res = run_bass_kernel_spmd(nc, in_maps, core_ids=list(range(NCORES)))
    grams = [res.results[c]["gram"] for c in range(NCORES)]
    return _ccc_from_grams(grams)
